# revision 19
# baseline (speedup 1.0000x reference)
"""Distributed Trainium2 Bass kernel for nn_AttentionCell (B=1, S=4096, D=1024, H=16).

Sharding: tensor-parallel over heads, 2 heads per core, paired (h, h+8) so RoPE's
rotate-half (which pairs model dims d and d+512, i.e. heads h and h+8) stays local
to a core. Per core:
  - QKV projection for its 128 channels (computed transposed: [ch, S]) from a
    host-staged transposed bf16 copy of x.
  - RoPE via host-staged cos/sin tables fused with the bias-add on the
    PSUM->SBUF drain.
  - Causal attention with scores computed transposed ([k, q] layout) so the
    PV matmul needs no on-chip transposes; softmax without max-subtraction
    (scores are O(1) here, exp cannot overflow); denominator via ones-columns
    appended to V (PV output rows 64:128); diagonal-crossing k-tiles are
    column-range-restricted so exp/QK/PV skip the fully-masked region.
  - One AllToAll (bounce slices staged per block as attention completes)
    switches to sequence-parallel, then the output projection + bias +
    LayerNorm run on this core's S/8 rows. (Chunked/warmup collectives and
    sub-256KB peer slices silently mis-route between core pairs on this
    runtime - only the single late AllToAll with [128,512]-bf16 peer slices
    is reliable.)
Host concatenates the 8 per-core row-slices into the full output.
"""
import os
import sys

sys.path.insert(0, "/opt/trn_rl_repo")

import numpy as np
import ml_dtypes

BF = ml_dtypes.bfloat16

DIM = 1024
H = 16
NCORES = 8
QB = 512          # query block (columns of transposed scores)
KT = 128          # key tile (partition dim of transposed scores)
NDIAG = QB // KT  # k-tiles crossing the causal diagonal per q block
GR = 128          # output-row granule for the chunked AllToAll
LN_EPS = 1e-5
ROPE_THETA = 10000.0


_built = {}


def _build(S, triv_ln=False):
    """Build + compile the 8-core SPMD graph for sequence length S."""
    import concourse.bass as bass
    import concourse.bacc as bacc
    import concourse.tile as tile
    import concourse.mybir as mybir

    f32 = mybir.dt.float32
    bf16 = mybir.dt.bfloat16
    AF = mybir.ActivationFunctionType
    OP = mybir.AluOpType

    assert S % 1024 == 0 and (S // NCORES) % 128 == 0
    SLC = S // NCORES          # output rows per core
    NQB = S // QB              # number of query blocks (= chunks)
    NKT = S // KT              # number of key tiles
    NCH = S // 512             # 512-wide chunks for projections (1 block each)
    NA2A = NQB // 2            # AllToAll chunks (2 blocks each)

    nc = bacc.Bacc("TRN2", target_bir_lowering=False, debug=False, num_devices=NCORES)

    xt_d = nc.dram_tensor("xt", [DIM, S], bf16, kind="ExternalInput").ap()
    wq_d = nc.dram_tensor("wq", [DIM, 128], bf16, kind="ExternalInput").ap()
    wk_d = nc.dram_tensor("wk", [DIM, 128], bf16, kind="ExternalInput").ap()
    wv_d = nc.dram_tensor("wv", [DIM, 128], bf16, kind="ExternalInput").ap()
    b3_d = nc.dram_tensor("b3", [3, 128, 1], f32, kind="ExternalInput").ap()
    cs_d = nc.dram_tensor("cs", [2, 128, S], bf16, kind="ExternalInput").ap()
    msk_d = nc.dram_tensor("msk", [128, 128], bf16, kind="ExternalInput").ap()
    id_d = nc.dram_tensor("ident", [128, 128], bf16, kind="ExternalInput").ap()
    wo_d = nc.dram_tensor("wo", [DIM, DIM], bf16, kind="ExternalInput").ap()
    bo16_d = nc.dram_tensor("bo16", [1, DIM], bf16, kind="ExternalInput").ap()
    lnc_d = nc.dram_tensor("lnc", [3, 128, DIM], f32, kind="ExternalInput").ap()
    out_d = nc.dram_tensor("out", [SLC, DIM], f32, kind="ExternalOutput").ap()

    with tile.TileContext(nc) as tc:
        with (
            tc.tile_pool(name="const", bufs=1) as cp,
            tc.tile_pool(name="dram", bufs=1, space="DRAM") as dramp,
        ):
            wq = cp.tile([128, 8, 128], bf16)
            wk = cp.tile([128, 8, 128], bf16)
            wv = cp.tile([128, 8, 128], bf16)
            b3 = cp.tile([128, 3], f32)
            tri = cp.tile([128, 128], bf16)     # tri[p, u] = (p <= u)
            ident = cp.tile([128, 128], bf16)
            wo = cp.tile([128, 8, DIM], bf16)
            lnc = cp.tile([128, 3, DIM], f32)
            bo16 = cp.tile([1, DIM], bf16)
            q_sbs = [cp.tile([128, 512], bf16, name=f"qsb{c}") for c in range(NCH)]
            k_sbs = [cp.tile([128, 512], bf16, name=f"ksb{c}") for c in range(NCH)]
            # per-k-tile [V_A(64) | ones(64) | V_B(64)]: the shared ones
            # plane gives both heads a contiguous 128-col PV lhsT
            # ([V_A|ones], [ones|V_B]) whose ones half emits the softmax
            # denominator (head A: ctx rows 64:128; head B: ctx rows 0:64).
            vall = cp.tile([128, NKT, 3, 64], bf16)
            ctxT = cp.tile([128, S], bf16)      # normalized ctx, [ch, q]
            ostage = cp.tile([128, SLC // 128, DIM], f32)  # Wo output staging
            mvs = cp.tile([128, SLC // 128, 2], f32)       # per-qtile LN mean/var
            epsc = cp.tile([128, 1], f32)
            ones1 = cp.tile([1, 128], bf16)

            # startup order matters: only wq + b3 + chunk-0 x/cos-sin gate the
            # first projection; everything else is emitted behind them
            for t in range(8):
                nc.sync.dma_start(wq[:, t, :], wq_d[128 * t:128 * (t + 1), :])
            for i in range(3):
                nc.sync.dma_start(b3[:, i:i + 1], b3_d[i])
            nc.vector.memset(vall[:, :, 1, :], 1.0)
            nc.vector.memset(epsc[:], LN_EPS)
            nc.vector.memset(ones1[:], 1.0)

            a2a_in = dramp.tile([NCORES, 128, SLC], bf16)
            a2a_out = dramp.tile([NCORES, 128, SLC], bf16)

            with (
                tc.tile_pool(name="p1", bufs=1) as p1,
                tc.tile_pool(name="p2", bufs=1) as p2,
                tc.tile_pool(name="ps_sc", bufs=2, space="PSUM") as pssc,
                tc.tile_pool(name="ps_pj", bufs=2, space="PSUM") as pspj,
                tc.tile_pool(name="ps_ctx", bufs=1, space="PSUM") as psctx,
            ):
                def proj_chunk(w_sb, b_i, dst, isrope, ch):
                    ps = pspj.tile([128, 512], f32, tag="pj", name="psproj")
                    for t in range(8):
                        nc.tensor.matmul(
                            ps[:], w_sb[:, t, :], xtc[ch % 2][:, t, :],
                            start=(t == 0), stop=(t == 7))
                    if isrope:
                        # rot = (p+b)*cos_dup + (p_swapped+b)*sin_signed
                        mA = p1.tile([128, 512], f32, tag="mA", bufs=2)
                        mB = p1.tile([128, 512], f32, tag="mB", bufs=2)
                        bq0 = b3[0:64, b_i:b_i + 1]
                        bq1 = b3[64:128, b_i:b_i + 1]
                        csc_c = cscs[ch % 2]
                        nc.vector.scalar_tensor_tensor(
                            mA[:], ps[:], b3[:, b_i:b_i + 1], csc_c[:, 0, :],
                            op0=OP.add, op1=OP.mult)
                        nc.vector.scalar_tensor_tensor(
                            mB[0:64, :], ps[64:128, :], bq1, csc_c[64:128, 1, :],
                            op0=OP.add, op1=OP.mult)
                        nc.vector.scalar_tensor_tensor(
                            mB[64:128, :], ps[0:64, :], bq0, csc_c[0:64, 1, :],
                            op0=OP.add, op1=OP.mult)
                        nc.vector.tensor_add(dst[:], mA[:], mB[:])
                    else:
                        nc.vector.tensor_scalar_add(vtss[ch % 2][:], ps[:], b3[:, 2:3])

                def transpose_chunk(ch):
                    # V^T via PE transpose-mode (the DMA xbar transpose
                    # mis-transposes on real HW despite passing CoreSim)
                    for j in range(4):
                        st = 4 * ch + j
                        tp = pspj.tile([128, 128], bf16, tag="pj",
                                       padded_shape=[128, 1024], name="pstr")
                        nc.tensor.transpose(
                            tp[:], vtss[ch % 2][:, 128 * j:128 * (j + 1)],
                            ident[:])
                        nc.vector.tensor_copy(
                            vall[:, st, 0:3:2, :],
                            tp[:].rearrange("p (g c) -> p g c", c=64))

                def emit_qk(qb, kt):
                    # one group = one 128-wide k-tile against the q block.
                    # head A scores land in bank 0 of the sc slot, head B in
                    # bank 1; lhsT base partitions 0/64 row-pack the two
                    # matmuls into concurrent PE row-groups.
                    r = kt - NDIAG * qb
                    c0 = 128 * r if r > 0 else 0
                    sc = pssc.tile([128, 2, QB], f32, tag="sc", name="scsc")
                    pt = p2.tile([128, 2, QB], bf16, tag="pt", bufs=4)
                    kch, ko = kt // 4, 128 * (kt % 4)
                    nc.tensor.matmul(
                        sc[:, 0, c0:], k_sbs[kch][0:64, ko:ko + 128],
                        q_sbs[qb][0:64, c0:], start=True, stop=True)
                    nc.tensor.matmul(
                        sc[:, 1, c0:], k_sbs[kch][64:128, ko:ko + 128],
                        q_sbs[qb][64:128, c0:], start=True, stop=True)
                    nc.scalar.activation(pt[:, :, c0:], sc[:, :, c0:],
                                         AF.Exp, scale=0.125)
                    return pt

                def emit_pv(qb, kt, first, last, pt, ctx):
                    nk = (QB * (qb + 1)) // KT
                    r = kt - NDIAG * qb
                    c0 = 128 * r if r > 0 else 0
                    if r >= 0:  # diagonal k-tile: causal-mask the partial strip
                        cs_ = slice(128 * r, 128 * r + 128)
                        nc.vector.tensor_mul(pt[:, 0, cs_], pt[:, 0, cs_], tri[:])
                        nc.vector.tensor_mul(pt[:, 1, cs_], pt[:, 1, cs_], tri[:])
                    nc.tensor.matmul(
                        ctx[:, 0, c0:], vall[:, kt, 0:2, :], pt[:, 0, c0:],
                        start=(kt == 0), stop=(kt == nk - 1))
                    nc.tensor.matmul(
                        ctx[:, 1, c0:], vall[:, kt, 1:3, :], pt[:, 1, c0:],
                        start=(kt == 0), stop=(kt == nk - 1))

                def emit_norm(qb, ctx):
                    # head A: ctx rows 0:64, denom rows 64:128 (plane 0)
                    # head B: denom rows 0:64, ctx rows 64:128 (plane 1)
                    # denominators bounce through SBUF: the custom-DVE approx
                    # reciprocal mis-reads PSUM operands on real HW
                    qs = slice(QB * qb, QB * (qb + 1))
                    den = p2.tile([64, 2, QB], f32, tag="den", bufs=2)
                    rb = p2.tile([64, 2, QB], f32, tag="rb", bufs=2)
                    nc.vector.tensor_copy(den[:, 0, :], ctx[64:128, 0, :])
                    nc.vector.tensor_copy(den[:, 1, :], ctx[0:64, 1, :])
                    nc.vector.reciprocal_approx_fast(rb[:], den[:])
                    nc.vector.tensor_mul(ctxT[0:64, qs], ctx[0:64, 0, :], rb[:, 0, :])
                    nc.vector.tensor_mul(ctxT[64:128, qs], ctx[64:128, 1, :], rb[:, 1, :])

                def emit_bounce(qb):
                    # stage block qb's normalized ctx for the AllToAll as soon
                    # as its norm lands (hides the bounce behind attention)
                    for j in range(qb * QB // SLC, (qb + 1) * QB // SLC):
                        nc.gpsimd.dma_start(
                            a2a_in[j], ctxT[:, SLC * j:SLC * (j + 1)])

                def emit_phase4(qt):
                    # Wo + bias + bn-stats for local rows [128*qt, 128*qt+128)
                    # of this core's SLC slice. Normalize/scale happens in the
                    # tail (sqrt needs a different ACT table set than exp).
                    stats = p1.tile([128, 2, 6], f32, tag="stats", bufs=2,
                                    name=f"stats{qt}")
                    tsl = slice(128 * qt, 128 * (qt + 1))
                    for o in range(2):
                        osl = slice(512 * o, 512 * (o + 1))
                        ops = pspj.tile([128, 512], f32, tag="pj", name="pswo")
                        for ct in range(8):
                            nc.tensor.matmul(
                                ops[:], ctxF[:, ct, tsl], wo[:, ct, osl],
                                start=(ct == 0), stop=False)
                        # rank-1 bias add closes the PSUM group
                        nc.tensor.matmul(
                            ops[:], ones1[:], bo16[:, osl],
                            start=False, stop=True)
                        nc.vector.bn_stats(stats[:, o, :], ops[:])
                        nc.vector.tensor_copy(ostage[:, qt, osl], ops[:])
                    nc.vector.bn_aggr(mvs[:, qt, :], stats[:])

                xtc = [p1.tile([128, 8, 512], bf16, tag=f"xtc{s}", name=f"xtc{s}")
                       for s in range(2)]
                cscs = [p1.tile([128, 2, 512], bf16, tag=f"csc{s}", name=f"csc{s}")
                        for s in range(2)]
                vtss = [p1.tile([128, 512], bf16, tag=f"vts{s}", name=f"vts{s}")
                        for s in range(2)]

                from collections import deque
                pending = deque()   # (qb, kt, first, last, pt, ctx)

                def flush_pending():
                    while pending:
                        d = pending.popleft()
                        emit_pv(*d)
                        if d[3]:
                            emit_norm(d[0], d[5])

                ctx = None
                for ch in range(NCH):
                    for t in range(8):
                        nc.sync.dma_start(
                            xtc[ch % 2][:, t, :],
                            xt_d[128 * t:128 * (t + 1), 512 * ch:512 * (ch + 1)])
                    for i in range(2):
                        nc.sync.dma_start(
                            cscs[ch % 2][:, i, :], cs_d[i, :, 512 * ch:512 * (ch + 1)])
                    if ch == 0:
                        # deferred constants: nothing here gates chunk 0's
                        # projection, so they queue behind its x/cos-sin loads
                        for t in range(8):
                            nc.sync.dma_start(wk[:, t, :],
                                              wk_d[128 * t:128 * (t + 1), :])
                        for t in range(8):
                            nc.sync.dma_start(wv[:, t, :],
                                              wv_d[128 * t:128 * (t + 1), :])
                        nc.sync.dma_start(tri[:], msk_d[:])
                        nc.sync.dma_start(ident[:], id_d[:])
                    if ch == 1:
                        # Wo / LN constants prefetch on the idle gpsimd queue
                        for t in range(8):
                            nc.gpsimd.dma_start(wo[:, t, :],
                                                wo_d[128 * t:128 * (t + 1), :])
                        nc.gpsimd.dma_start(bo16[:], bo16_d[:])
                        if not triv_ln:
                            for i in range(3):
                                nc.gpsimd.dma_start(lnc[:, i, :], lnc_d[i])

                    proj_chunk(wq, 0, q_sbs[ch], True, ch)
                    # drain leftovers of the previous block: PE gets PV work
                    # while the DVE computes this chunk's RoPE
                    flush_pending()
                    if ch >= 1:
                        emit_bounce(ch - 1)
                    proj_chunk(wk, 1, k_sbs[ch], True, ch)
                    proj_chunk(wv, 2, None, False, ch)
                    transpose_chunk(ch)

                    qb = ch
                    nk = (QB * (qb + 1)) // KT
                    for kt in range(nk):
                        if kt == 0:
                            ctx = psctx.tile([128, 2, QB], f32, tag="ctx")
                        pt = emit_qk(qb, kt)
                        pending.append((qb, kt, kt == 0, kt == nk - 1, pt, ctx))
                        if len(pending) > 2:
                            d = pending.popleft()
                            emit_pv(*d)
                            if d[3]:
                                emit_norm(d[0], d[5])

                flush_pending()
                emit_bounce(NQB - 1)
                nc.gpsimd.collective_compute(
                    "AllToAll",
                    mybir.AluOpType.bypass,
                    replica_groups=[list(range(NCORES))],
                    ins=[a2a_in[:].opt()],
                    outs=[a2a_out[:].opt()],
                )
                ctxF = p1.tile([128, NCORES, SLC], bf16)
                for j in range(NCORES):
                    nc.sync.dma_start(ctxF[:, j, :], a2a_out[j])

                # ─── LN tail: sqrt (one ACT table switch) + scale + store.
                # Chunks 0..NA2A-2 are emitted before the last phase4 so the
                # ACT table switch and their stores hide under the final
                # AllToAll; the last chunk follows its phase4. ───
                def emit_ln_tail(m):
                    sd = p1.tile([128, 2], f32, tag="sd", bufs=2, name=f"sd{m}")
                    nc.scalar.activation(sd[:, 0:1], mvs[:, m, 1:2],
                                         AF.Sqrt, bias=epsc[:])
                    nc.vector.reciprocal(sd[:, 1:2], sd[:, 0:1])
                    tsl = slice(128 * m, 128 * (m + 1))
                    if triv_ln:
                        t2 = p1.tile([128, DIM], f32, tag="t2", bufs=2,
                                     name=f"t2{m}")
                        nc.vector.tensor_scalar(
                            t2[:], ostage[:, m, :], mvs[:, m, 0:1], sd[:, 1:2],
                            op0=OP.subtract, op1=OP.mult)
                        nc.sync.dma_start(out_d[tsl, :], t2[:])
                    else:
                        t2 = p1.tile([128, DIM], f32, tag="t2", bufs=2,
                                     name=f"t2{m}")
                        nc.vector.tensor_scalar(
                            t2[:], ostage[:, m, :], mvs[:, m, 0:1], sd[:, 1:2],
                            op0=OP.subtract, op1=OP.mult)
                        t3 = p1.tile([128, DIM], f32, tag="t3", bufs=2,
                                     name=f"t3{m}")
                        nc.vector.tensor_mul(t3[:], t2[:], lnc[:, 1, :])
                        ob = p1.tile([128, DIM], f32, tag="ob", bufs=2,
                                     name=f"ob{m}")
                        nc.vector.tensor_add(ob[:], t3[:], lnc[:, 2, :])
                        nc.sync.dma_start(out_d[tsl, :], ob[:])

                for qt in range(SLC // 128):
                    emit_phase4(qt)
                for qt in range(SLC // 128):
                    emit_ln_tail(qt)

    nc.compile()
    return nc


def get_nc(S=4096, triv_ln=False):
    key = (S, triv_ln)
    if key not in _built:
        _built[key] = _build(S, triv_ln)
    return _built[key]


def stage_inputs(x, Wqkv, bqkv, Wo, bo, gamma, beta):
    """Host-side sharding/staging. Returns in_maps for the 8 cores."""
    x = np.asarray(x, dtype=np.float32)
    Wqkv = np.asarray(Wqkv, dtype=np.float32)
    bqkv = np.asarray(bqkv, dtype=np.float32)
    Wo = np.asarray(Wo, dtype=np.float32)
    bo = np.asarray(bo, dtype=np.float32)
    gamma = np.asarray(gamma, dtype=np.float32)
    beta = np.asarray(beta, dtype=np.float32)

    S = x.shape[1]
    xt = np.ascontiguousarray(x[0].T).astype(BF)                       # [DIM, S]
    inv_freq = 1.0 / (ROPE_THETA ** (np.arange(0, DIM, 2, dtype=np.float64) / DIM))

    # Wo rows permuted to the post-AllToAll channel order
    perm = np.concatenate([
        np.concatenate([np.arange(64 * j, 64 * j + 64),
                        np.arange(512 + 64 * j, 512 + 64 * j + 64)])
        for j in range(NCORES)
    ])
    wo = np.ascontiguousarray(Wo[perm, :]).astype(BF)

    p = np.arange(128)[:, None]
    u = np.arange(128)[None, :]
    tri = (p <= u).astype(BF)                     # causal mask for diag strips
    ident = np.eye(128, dtype=np.float32).astype(BF)
    lnc = np.stack([
        np.broadcast_to(bo, (128, DIM)),
        np.broadcast_to(gamma, (128, DIM)),
        np.broadcast_to(beta, (128, DIM)),
    ]).astype(np.float32)

    in_maps = []
    for c in range(NCORES):
        cols = np.concatenate([np.arange(64 * c, 64 * c + 64),
                               np.arange(512 + 64 * c, 512 + 64 * c + 64)])
        ang = np.arange(S, dtype=np.float64)[None, :] * inv_freq[64 * c:64 * c + 64][:, None]
        C = np.cos(ang)
        Sn = np.sin(ang)
        # plane 0: cos duplicated; plane 1: +sin rows 0:64, -sin rows 64:128
        # (the sign flip folds the rotate-half subtraction into one tensor_add)
        cs = np.stack([np.concatenate([C, C], 0),
                       np.concatenate([Sn, -Sn], 0)]).astype(BF)        # [2,128,S]
        b3 = np.stack([bqkv[cols], bqkv[1024 + cols], bqkv[2048 + cols]]
                      ).astype(np.float32)[:, :, None]                  # [3,128,1]
        in_maps.append({
            "xt": xt,
            "wq": np.ascontiguousarray(Wqkv[:, cols]).astype(BF),
            "wk": np.ascontiguousarray(Wqkv[:, 1024 + cols]).astype(BF),
            "wv": np.ascontiguousarray(Wqkv[:, 2048 + cols]).astype(BF),
            "b3": b3,
            "cs": cs,
            "msk": tri,
            "ident": ident,
            "wo": wo,
            "bo16": bo.reshape(1, DIM).astype(BF),
            "lnc": lnc,
        })
    return in_maps


def gather_out(outs, S):
    """Reassemble the full [1, S, DIM] output: core c owns rows
    [S/8*c : S/8*(c+1))."""
    return np.concatenate(outs, axis=0)[None]


def kernel(x, Wqkv, bqkv, Wo, bo, gamma, beta):
    from concourse import bass_utils

    x = np.asarray(x)
    S = x.shape[1]
    triv = bool(np.all(np.asarray(gamma) == 1.0) and np.all(np.asarray(beta) == 0.0))
    nc = get_nc(S, triv)
    in_maps = stage_inputs(x, Wqkv, bqkv, Wo, bo, gamma, beta)
    res = bass_utils.run_bass_kernel_spmd(nc, in_maps, core_ids=list(range(NCORES)))
    return gather_out([res.results[c]["out"] for c in range(NCORES)], S).astype(
        np.float32)


# revision 20
# speedup vs baseline: 1.0274x; 1.0274x over previous
"""Distributed Trainium2 Bass kernel for nn_AttentionCell (B=1, S=4096, D=1024, H=16).

Sharding: tensor-parallel over heads, 2 heads per core, paired (h, h+8) so RoPE's
rotate-half (which pairs model dims d and d+512, i.e. heads h and h+8) stays local
to a core. Per core:
  - QKV projection for its 128 channels (computed transposed: [ch, S]) from a
    host-staged transposed bf16 copy of x.
  - RoPE via host-staged cos/sin tables fused with the bias-add on the
    PSUM->SBUF drain.
  - Causal attention with scores computed transposed ([k, q] layout) so the
    PV matmul needs no on-chip transposes; softmax without max-subtraction
    (scores are O(1) here, exp cannot overflow); denominator via ones-columns
    appended to V (PV output rows 64:128); diagonal-crossing k-tiles are
    column-range-restricted so exp/QK/PV skip the fully-masked region.
  - One AllToAll (bounce slices staged per block as attention completes)
    switches to sequence-parallel, then the output projection + bias +
    LayerNorm run on this core's S/8 rows. (Chunked/warmup collectives and
    sub-256KB peer slices silently mis-route between core pairs on this
    runtime - only the single late AllToAll with [128,512]-bf16 peer slices
    is reliable.)
Host concatenates the 8 per-core row-slices into the full output.
"""
import os
import sys

sys.path.insert(0, "/opt/trn_rl_repo")

import numpy as np
import ml_dtypes

BF = ml_dtypes.bfloat16

DIM = 1024
H = 16
NCORES = 8
QB = 512          # query block (columns of transposed scores)
KT = 128          # key tile (partition dim of transposed scores)
NDIAG = QB // KT  # k-tiles crossing the causal diagonal per q block
GR = 128          # output-row granule for the chunked AllToAll
LN_EPS = 1e-5
ROPE_THETA = 10000.0


_built = {}


def _build(S, triv_ln=False):
    """Build + compile the 8-core SPMD graph for sequence length S."""
    import concourse.bass as bass
    import concourse.bacc as bacc
    import concourse.tile as tile
    import concourse.mybir as mybir

    f32 = mybir.dt.float32
    bf16 = mybir.dt.bfloat16
    AF = mybir.ActivationFunctionType
    OP = mybir.AluOpType

    assert S % 1024 == 0 and (S // NCORES) % 128 == 0
    SLC = S // NCORES          # output rows per core
    NQB = S // QB              # number of query blocks (= chunks)
    NKT = S // KT              # number of key tiles
    NCH = S // 512             # 512-wide chunks for projections (1 block each)
    NA2A = NQB // 2            # AllToAll chunks (2 blocks each)

    nc = bacc.Bacc("TRN2", target_bir_lowering=False, debug=False, num_devices=NCORES)

    xt_d = nc.dram_tensor("xt", [DIM, S], bf16, kind="ExternalInput").ap()
    wq_d = nc.dram_tensor("wq", [DIM, 128], bf16, kind="ExternalInput").ap()
    wk_d = nc.dram_tensor("wk", [DIM, 128], bf16, kind="ExternalInput").ap()
    wv_d = nc.dram_tensor("wv", [DIM, 128], bf16, kind="ExternalInput").ap()
    b3_d = nc.dram_tensor("b3", [3, 128, 1], f32, kind="ExternalInput").ap()
    cs_d = nc.dram_tensor("cs", [2, 128, S], bf16, kind="ExternalInput").ap()
    msk_d = nc.dram_tensor("msk", [128, 128], bf16, kind="ExternalInput").ap()
    id_d = nc.dram_tensor("ident", [128, 128], bf16, kind="ExternalInput").ap()
    wo_d = nc.dram_tensor("wo", [DIM, DIM], bf16, kind="ExternalInput").ap()
    bo16_d = nc.dram_tensor("bo16", [1, DIM], bf16, kind="ExternalInput").ap()
    lnc_d = nc.dram_tensor("lnc", [3, 128, DIM], f32, kind="ExternalInput").ap()
    out_d = nc.dram_tensor("out", [SLC, DIM], f32, kind="ExternalOutput").ap()

    with tile.TileContext(nc) as tc:
        with (
            tc.tile_pool(name="const", bufs=1) as cp,
            tc.tile_pool(name="dram", bufs=1, space="DRAM") as dramp,
        ):
            wq = cp.tile([128, 8, 128], bf16)
            wk = cp.tile([128, 8, 128], bf16)
            wv = cp.tile([128, 8, 128], bf16)
            b3 = cp.tile([128, 3], f32)
            tri = cp.tile([128, 128], bf16)     # tri[p, u] = (p <= u)
            ident = cp.tile([128, 128], bf16)
            wo = cp.tile([128, 8, DIM], bf16)
            lnc = cp.tile([128, 3, DIM], f32)
            bo16 = cp.tile([1, DIM], bf16)
            q_sbs = [cp.tile([128, 512], bf16, name=f"qsb{c}") for c in range(NCH)]
            k_sbs = [cp.tile([128, 512], bf16, name=f"ksb{c}") for c in range(NCH)]
            # per-k-tile [V_A(64) | ones(64) | V_B(64)]: the shared ones
            # plane gives both heads a contiguous 128-col PV lhsT
            # ([V_A|ones], [ones|V_B]) whose ones half emits the softmax
            # denominator (head A: ctx rows 64:128; head B: ctx rows 0:64).
            vall = cp.tile([128, NKT, 3, 64], bf16)
            ctxT = cp.tile([128, S], bf16)      # normalized ctx, [ch, q]
            mvs = cp.tile([128, SLC // 128, 2], f32)       # per-qtile LN mean/var
            epsc = cp.tile([128, 1], f32)
            ones1 = cp.tile([1, 128], bf16)

            # startup order matters: only wq + b3 + chunk-0 x/cos-sin gate the
            # first projection; everything else is emitted behind them
            for t in range(8):
                nc.sync.dma_start(wq[:, t, :], wq_d[128 * t:128 * (t + 1), :])
            for i in range(3):
                nc.sync.dma_start(b3[:, i:i + 1], b3_d[i])
            nc.vector.memset(vall[:, :, 1, :], 1.0)
            nc.vector.memset(epsc[:], LN_EPS)
            nc.vector.memset(ones1[:], 1.0)

            a2a_in = dramp.tile([NCORES, 128, SLC], bf16)
            a2a_out = dramp.tile([NCORES, 128, SLC], bf16)

            with (
                tc.tile_pool(name="p1", bufs=1) as p1,
                tc.tile_pool(name="p2", bufs=1) as p2,
                tc.tile_pool(name="ps_sc", bufs=2, space="PSUM") as pssc,
                tc.tile_pool(name="ps_pj", bufs=2, space="PSUM") as pspj,
                tc.tile_pool(name="ps_ctx", bufs=1, space="PSUM") as psctx,
            ):
                def proj_chunk(w_sb, b_i, dst, isrope, ch):
                    ps = pspj.tile([128, 512], f32, tag="pj", name="psproj")
                    for t in range(8):
                        nc.tensor.matmul(
                            ps[:], w_sb[:, t, :], xtc[ch % 2][:, t, :],
                            start=(t == 0), stop=(t == 7))
                    if isrope:
                        # rot = (p+b)*cos_dup + (p_swapped+b)*sin_signed
                        mA = p1.tile([128, 512], f32, tag="mA", bufs=2)
                        mB = p1.tile([128, 512], f32, tag="mB", bufs=2)
                        bq0 = b3[0:64, b_i:b_i + 1]
                        bq1 = b3[64:128, b_i:b_i + 1]
                        csc_c = cscs[ch % 2]
                        nc.vector.scalar_tensor_tensor(
                            mA[:], ps[:], b3[:, b_i:b_i + 1], csc_c[:, 0, :],
                            op0=OP.add, op1=OP.mult)
                        nc.vector.scalar_tensor_tensor(
                            mB[0:64, :], ps[64:128, :], bq1, csc_c[64:128, 1, :],
                            op0=OP.add, op1=OP.mult)
                        nc.vector.scalar_tensor_tensor(
                            mB[64:128, :], ps[0:64, :], bq0, csc_c[0:64, 1, :],
                            op0=OP.add, op1=OP.mult)
                        nc.vector.tensor_add(dst[:], mA[:], mB[:])
                    else:
                        nc.vector.tensor_scalar_add(vtss[ch % 2][:], ps[:], b3[:, 2:3])

                def transpose_chunk(ch):
                    # V^T via PE transpose-mode (the DMA xbar transpose
                    # mis-transposes on real HW despite passing CoreSim)
                    for j in range(4):
                        st = 4 * ch + j
                        tp = pspj.tile([128, 128], bf16, tag="pj",
                                       padded_shape=[128, 1024], name="pstr")
                        nc.tensor.transpose(
                            tp[:], vtss[ch % 2][:, 128 * j:128 * (j + 1)],
                            ident[:])
                        nc.vector.tensor_copy(
                            vall[:, st, 0:3:2, :],
                            tp[:].rearrange("p (g c) -> p g c", c=64))

                def emit_qk(qb, kt):
                    # one group = one 128-wide k-tile against the q block.
                    # head A scores land in bank 0 of the sc slot, head B in
                    # bank 1; lhsT base partitions 0/64 row-pack the two
                    # matmuls into concurrent PE row-groups.
                    r = kt - NDIAG * qb
                    c0 = 128 * r if r > 0 else 0
                    sc = pssc.tile([128, 2, QB], f32, tag="sc", name="scsc")
                    pt = p2.tile([128, 2, QB], bf16, tag="pt", bufs=4)
                    kch, ko = kt // 4, 128 * (kt % 4)
                    nc.tensor.matmul(
                        sc[:, 0, c0:], k_sbs[kch][0:64, ko:ko + 128],
                        q_sbs[qb][0:64, c0:], start=True, stop=True)
                    nc.tensor.matmul(
                        sc[:, 1, c0:], k_sbs[kch][64:128, ko:ko + 128],
                        q_sbs[qb][64:128, c0:], start=True, stop=True)
                    nc.scalar.activation(pt[:, :, c0:], sc[:, :, c0:],
                                         AF.Exp, scale=0.125)
                    return pt

                def emit_pv(qb, kt, first, last, pt, ctx):
                    nk = (QB * (qb + 1)) // KT
                    r = kt - NDIAG * qb
                    c0 = 128 * r if r > 0 else 0
                    if r >= 0:  # diagonal k-tile: causal-mask the partial strip
                        cs_ = slice(128 * r, 128 * r + 128)
                        nc.vector.tensor_mul(pt[:, 0, cs_], pt[:, 0, cs_], tri[:])
                        nc.vector.tensor_mul(pt[:, 1, cs_], pt[:, 1, cs_], tri[:])
                    nc.tensor.matmul(
                        ctx[:, 0, c0:], vall[:, kt, 0:2, :], pt[:, 0, c0:],
                        start=(kt == 0), stop=(kt == nk - 1))
                    nc.tensor.matmul(
                        ctx[:, 1, c0:], vall[:, kt, 1:3, :], pt[:, 1, c0:],
                        start=(kt == 0), stop=(kt == nk - 1))

                def emit_norm(qb, ctx):
                    # head A: ctx rows 0:64, denom rows 64:128 (plane 0)
                    # head B: denom rows 0:64, ctx rows 64:128 (plane 1)
                    # denominators bounce through SBUF: the custom-DVE approx
                    # reciprocal mis-reads PSUM operands on real HW
                    qs = slice(QB * qb, QB * (qb + 1))
                    den = p2.tile([64, 2, QB], f32, tag="den", bufs=2)
                    rb = p2.tile([64, 2, QB], f32, tag="rb", bufs=2)
                    nc.vector.tensor_copy(den[:, 0, :], ctx[64:128, 0, :])
                    nc.vector.tensor_copy(den[:, 1, :], ctx[0:64, 1, :])
                    nc.vector.reciprocal_approx_fast(rb[:], den[:])
                    nc.vector.tensor_mul(ctxT[0:64, qs], ctx[0:64, 0, :], rb[:, 0, :])
                    nc.vector.tensor_mul(ctxT[64:128, qs], ctx[64:128, 1, :], rb[:, 1, :])

                def emit_bounce(qb):
                    # stage block qb's normalized ctx for the AllToAll as soon
                    # as its norm lands (hides the bounce behind attention)
                    for j in range(qb * QB // SLC, (qb + 1) * QB // SLC):
                        nc.gpsimd.dma_start(
                            a2a_in[j], ctxT[:, SLC * j:SLC * (j + 1)])

                def emit_phase4(qt):
                    # Wo + bias + bn-stats + LayerNorm for local rows
                    # [128*qt, 128*qt+128), normalized straight out of the two
                    # live PSUM slots (no SBUF staging copy).
                    stats = p1.tile([128, 2, 6], f32, tag="stats", bufs=2,
                                    name=f"stats{qt}")
                    tsl = slice(128 * qt, 128 * (qt + 1))
                    opss = []
                    for o in range(2):
                        osl = slice(512 * o, 512 * (o + 1))
                        ops = pspj.tile([128, 512], f32, tag="pj", name="pswo")
                        opss.append(ops)
                        for ct in range(8):
                            nc.tensor.matmul(
                                ops[:], ctxF[:, ct, tsl], wo[:, ct, osl],
                                start=(ct == 0), stop=False)
                        # rank-1 bias add closes the PSUM group
                        nc.tensor.matmul(
                            ops[:], ones1[:], bo16[:, osl],
                            start=False, stop=True)
                        nc.vector.bn_stats(stats[:, o, :], ops[:])
                    nc.vector.bn_aggr(mvs[:, qt, :], stats[:])
                    sd = p1.tile([128, 2], f32, tag="sd", bufs=2,
                                 name=f"sd{qt}")
                    nc.scalar.activation(sd[:, 0:1], mvs[:, qt, 1:2],
                                         AF.Sqrt, bias=epsc[:])
                    nc.vector.reciprocal(sd[:, 1:2], sd[:, 0:1])
                    t2 = p1.tile([128, DIM], f32, tag="t2", bufs=2,
                                 name=f"t2{qt}")
                    for o in range(2):
                        osl = slice(512 * o, 512 * (o + 1))
                        nc.vector.tensor_scalar(
                            t2[:, osl], opss[o][:], mvs[:, qt, 0:1], sd[:, 1:2],
                            op0=OP.subtract, op1=OP.mult)
                    if triv_ln:
                        nc.sync.dma_start(out_d[tsl, :], t2[:])
                    else:
                        t3 = p1.tile([128, DIM], f32, tag="t3", bufs=2,
                                     name=f"t3{qt}")
                        nc.vector.tensor_mul(t3[:], t2[:], lnc[:, 1, :])
                        ob = p1.tile([128, DIM], f32, tag="ob", bufs=2,
                                     name=f"ob{qt}")
                        nc.vector.tensor_add(ob[:], t3[:], lnc[:, 2, :])
                        nc.sync.dma_start(out_d[tsl, :], ob[:])

                xtc = [p1.tile([128, 8, 512], bf16, tag=f"xtc{s}", name=f"xtc{s}")
                       for s in range(2)]
                cscs = [p1.tile([128, 2, 512], bf16, tag=f"csc{s}", name=f"csc{s}")
                        for s in range(2)]
                vtss = [p1.tile([128, 512], bf16, tag=f"vts{s}", name=f"vts{s}")
                        for s in range(2)]

                from collections import deque
                pending = deque()   # (qb, kt, first, last, pt, ctx)

                def flush_pending():
                    while pending:
                        d = pending.popleft()
                        emit_pv(*d)
                        if d[3]:
                            emit_norm(d[0], d[5])

                ctx = None
                for ch in range(NCH):
                    for t in range(8):
                        eng = nc.sync if t % 2 == 0 else nc.scalar
                        eng.dma_start(
                            xtc[ch % 2][:, t, :],
                            xt_d[128 * t:128 * (t + 1), 512 * ch:512 * (ch + 1)])
                    for i in range(2):
                        nc.sync.dma_start(
                            cscs[ch % 2][:, i, :], cs_d[i, :, 512 * ch:512 * (ch + 1)])
                    if ch == 0:
                        # deferred constants: nothing here gates chunk 0's
                        # projection, so they queue behind its x/cos-sin loads
                        for t in range(8):
                            nc.sync.dma_start(wk[:, t, :],
                                              wk_d[128 * t:128 * (t + 1), :])
                        for t in range(8):
                            nc.sync.dma_start(wv[:, t, :],
                                              wv_d[128 * t:128 * (t + 1), :])
                        nc.sync.dma_start(tri[:], msk_d[:])
                        nc.sync.dma_start(ident[:], id_d[:])
                    if ch == 1:
                        # Wo / LN constants prefetch on the idle gpsimd queue
                        for t in range(8):
                            nc.gpsimd.dma_start(wo[:, t, :],
                                                wo_d[128 * t:128 * (t + 1), :])
                        nc.gpsimd.dma_start(bo16[:], bo16_d[:])
                        if not triv_ln:
                            for i in range(3):
                                nc.gpsimd.dma_start(lnc[:, i, :], lnc_d[i])

                    proj_chunk(wq, 0, q_sbs[ch], True, ch)
                    # drain leftovers of the previous block: PE gets PV work
                    # while the DVE computes this chunk's RoPE
                    flush_pending()
                    if ch >= 1:
                        emit_bounce(ch - 1)
                    proj_chunk(wk, 1, k_sbs[ch], True, ch)
                    proj_chunk(wv, 2, None, False, ch)
                    transpose_chunk(ch)

                    qb = ch
                    nk = (QB * (qb + 1)) // KT
                    for kt in range(nk):
                        if kt == 0:
                            ctx = psctx.tile([128, 2, QB], f32, tag="ctx")
                        pt = emit_qk(qb, kt)
                        pending.append((qb, kt, kt == 0, kt == nk - 1, pt, ctx))
                        if len(pending) > 2:
                            d = pending.popleft()
                            emit_pv(*d)
                            if d[3]:
                                emit_norm(d[0], d[5])

                flush_pending()
                emit_bounce(NQB - 1)
                nc.gpsimd.collective_compute(
                    "AllToAll",
                    mybir.AluOpType.bypass,
                    replica_groups=[list(range(NCORES))],
                    ins=[a2a_in[:].opt()],
                    outs=[a2a_out[:].opt()],
                )
                ctxF = p1.tile([128, NCORES, SLC], bf16)
                for j in range(NCORES):
                    nc.sync.dma_start(ctxF[:, j, :], a2a_out[j])

                for qt in range(SLC // 128):
                    emit_phase4(qt)

    nc.compile()
    return nc


def get_nc(S=4096, triv_ln=False):
    key = (S, triv_ln)
    if key not in _built:
        _built[key] = _build(S, triv_ln)
    return _built[key]


def stage_inputs(x, Wqkv, bqkv, Wo, bo, gamma, beta):
    """Host-side sharding/staging. Returns in_maps for the 8 cores."""
    x = np.asarray(x, dtype=np.float32)
    Wqkv = np.asarray(Wqkv, dtype=np.float32)
    bqkv = np.asarray(bqkv, dtype=np.float32)
    Wo = np.asarray(Wo, dtype=np.float32)
    bo = np.asarray(bo, dtype=np.float32)
    gamma = np.asarray(gamma, dtype=np.float32)
    beta = np.asarray(beta, dtype=np.float32)

    S = x.shape[1]
    xt = np.ascontiguousarray(x[0].T).astype(BF)                       # [DIM, S]
    inv_freq = 1.0 / (ROPE_THETA ** (np.arange(0, DIM, 2, dtype=np.float64) / DIM))

    # Wo rows permuted to the post-AllToAll channel order
    perm = np.concatenate([
        np.concatenate([np.arange(64 * j, 64 * j + 64),
                        np.arange(512 + 64 * j, 512 + 64 * j + 64)])
        for j in range(NCORES)
    ])
    wo = np.ascontiguousarray(Wo[perm, :]).astype(BF)

    p = np.arange(128)[:, None]
    u = np.arange(128)[None, :]
    tri = (p <= u).astype(BF)                     # causal mask for diag strips
    ident = np.eye(128, dtype=np.float32).astype(BF)
    lnc = np.stack([
        np.broadcast_to(bo, (128, DIM)),
        np.broadcast_to(gamma, (128, DIM)),
        np.broadcast_to(beta, (128, DIM)),
    ]).astype(np.float32)

    in_maps = []
    for c in range(NCORES):
        cols = np.concatenate([np.arange(64 * c, 64 * c + 64),
                               np.arange(512 + 64 * c, 512 + 64 * c + 64)])
        ang = np.arange(S, dtype=np.float64)[None, :] * inv_freq[64 * c:64 * c + 64][:, None]
        C = np.cos(ang)
        Sn = np.sin(ang)
        # plane 0: cos duplicated; plane 1: +sin rows 0:64, -sin rows 64:128
        # (the sign flip folds the rotate-half subtraction into one tensor_add)
        cs = np.stack([np.concatenate([C, C], 0),
                       np.concatenate([Sn, -Sn], 0)]).astype(BF)        # [2,128,S]
        b3 = np.stack([bqkv[cols], bqkv[1024 + cols], bqkv[2048 + cols]]
                      ).astype(np.float32)[:, :, None]                  # [3,128,1]
        in_maps.append({
            "xt": xt,
            "wq": np.ascontiguousarray(Wqkv[:, cols]).astype(BF),
            "wk": np.ascontiguousarray(Wqkv[:, 1024 + cols]).astype(BF),
            "wv": np.ascontiguousarray(Wqkv[:, 2048 + cols]).astype(BF),
            "b3": b3,
            "cs": cs,
            "msk": tri,
            "ident": ident,
            "wo": wo,
            "bo16": bo.reshape(1, DIM).astype(BF),
            "lnc": lnc,
        })
    return in_maps


def gather_out(outs, S):
    """Reassemble the full [1, S, DIM] output: core c owns rows
    [S/8*c : S/8*(c+1))."""
    return np.concatenate(outs, axis=0)[None]


def kernel(x, Wqkv, bqkv, Wo, bo, gamma, beta):
    from concourse import bass_utils

    x = np.asarray(x)
    S = x.shape[1]
    triv = bool(np.all(np.asarray(gamma) == 1.0) and np.all(np.asarray(beta) == 0.0))
    nc = get_nc(S, triv)
    in_maps = stage_inputs(x, Wqkv, bqkv, Wo, bo, gamma, beta)
    res = bass_utils.run_bass_kernel_spmd(nc, in_maps, core_ids=list(range(NCORES)))
    return gather_out([res.results[c]["out"] for c in range(NCORES)], S).astype(
        np.float32)


# revision 21
# speedup vs baseline: 1.0604x; 1.0321x over previous
"""Distributed Trainium2 Bass kernel for nn_AttentionCell (B=1, S=4096, D=1024, H=16).

Sharding: tensor-parallel over heads, 2 heads per core, paired (h, h+8) so RoPE's
rotate-half (which pairs model dims d and d+512, i.e. heads h and h+8) stays local
to a core. Per core:
  - QKV projection for its 128 channels (computed transposed: [ch, S]) from a
    host-staged transposed bf16 copy of x.
  - RoPE via host-staged cos/sin tables fused with the bias-add on the
    PSUM->SBUF drain.
  - Causal attention with scores computed transposed ([k, q] layout) so the
    PV matmul needs no on-chip transposes; softmax without max-subtraction
    (scores are O(1) here, exp cannot overflow); denominator via ones-columns
    appended to V (PV output rows 64:128); diagonal-crossing k-tiles are
    column-range-restricted so exp/QK/PV skip the fully-masked region.
  - One AllToAll (bounce slices staged per block as attention completes)
    switches to sequence-parallel, then the output projection + bias +
    LayerNorm run on this core's S/8 rows. (Chunked/warmup collectives and
    sub-256KB peer slices silently mis-route between core pairs on this
    runtime - only the single late AllToAll with [128,512]-bf16 peer slices
    is reliable.)
Host concatenates the 8 per-core row-slices into the full output.
"""
import os
import sys

sys.path.insert(0, "/opt/trn_rl_repo")

import numpy as np
import ml_dtypes

BF = ml_dtypes.bfloat16

DIM = 1024
H = 16
NCORES = 8
QB = 512          # query block (columns of transposed scores)
KT = 128          # key tile (partition dim of transposed scores)
NDIAG = QB // KT  # k-tiles crossing the causal diagonal per q block
GR = 128          # output-row granule for the chunked AllToAll
LN_EPS = 1e-5
ROPE_THETA = 10000.0


_built = {}


def _build(S, triv_ln=False):
    """Build + compile the 8-core SPMD graph for sequence length S."""
    import concourse.bass as bass
    import concourse.bacc as bacc
    import concourse.tile as tile
    import concourse.mybir as mybir

    f32 = mybir.dt.float32
    bf16 = mybir.dt.bfloat16
    AF = mybir.ActivationFunctionType
    OP = mybir.AluOpType

    assert S % 1024 == 0 and (S // NCORES) % 128 == 0
    SLC = S // NCORES          # output rows per core
    NQB = S // QB              # number of query blocks (= chunks)
    NKT = S // KT              # number of key tiles
    NCH = S // 512             # 512-wide chunks for projections (1 block each)
    NA2A = NQB // 2            # AllToAll chunks (2 blocks each)

    nc = bacc.Bacc("TRN2", target_bir_lowering=False, debug=False, num_devices=NCORES)

    xt_d = nc.dram_tensor("xt", [DIM, S], bf16, kind="ExternalInput").ap()
    wq_d = nc.dram_tensor("wq", [DIM, 128], bf16, kind="ExternalInput").ap()
    wk_d = nc.dram_tensor("wk", [DIM, 128], bf16, kind="ExternalInput").ap()
    wv_d = nc.dram_tensor("wv", [DIM, 128], bf16, kind="ExternalInput").ap()
    b3_d = nc.dram_tensor("b3", [3, 128, 1], f32, kind="ExternalInput").ap()
    cs_d = nc.dram_tensor("cs", [2, 128, S], bf16, kind="ExternalInput").ap()
    msk_d = nc.dram_tensor("msk", [128, 128], bf16, kind="ExternalInput").ap()
    id_d = nc.dram_tensor("ident", [128, 128], bf16, kind="ExternalInput").ap()
    wo_d = nc.dram_tensor("wo", [DIM, DIM], bf16, kind="ExternalInput").ap()
    bo16_d = nc.dram_tensor("bo16", [1, DIM], bf16, kind="ExternalInput").ap()
    lnc_d = nc.dram_tensor("lnc", [3, 128, DIM], f32, kind="ExternalInput").ap()
    out_d = nc.dram_tensor("out", [SLC, DIM], f32, kind="ExternalOutput").ap()

    with tile.TileContext(nc) as tc:
        with (
            tc.tile_pool(name="const", bufs=1) as cp,
            tc.tile_pool(name="dram", bufs=1, space="DRAM") as dramp,
        ):
            wq = cp.tile([128, 8, 128], bf16)
            wk = cp.tile([128, 8, 128], bf16)
            wv = cp.tile([128, 8, 128], bf16)
            b3 = cp.tile([128, 3], f32)
            tri = cp.tile([128, 128], bf16)     # tri[p, u] = (p <= u)
            ident = cp.tile([128, 128], bf16)
            wo = cp.tile([128, 8, DIM], bf16)
            lnc = cp.tile([128, 3, DIM], f32)
            bo16 = cp.tile([1, DIM], bf16)
            q_sbs = [cp.tile([128, 512], bf16, name=f"qsb{c}") for c in range(NCH)]
            k_sbs = [cp.tile([128, 512], bf16, name=f"ksb{c}") for c in range(NCH)]
            # per-k-tile [V_A(64) | ones(64) | V_B(64)]: the shared ones
            # plane gives both heads a contiguous 128-col PV lhsT
            # ([V_A|ones], [ones|V_B]) whose ones half emits the softmax
            # denominator (head A: ctx rows 64:128; head B: ctx rows 0:64).
            vall = cp.tile([128, NKT, 3, 64], bf16)
            ctxT = cp.tile([128, S], bf16)      # normalized ctx, [ch, q]
            mvs = cp.tile([128, SLC // 128, 2], f32)       # per-qtile LN mean/var
            epsc = cp.tile([128, 1], f32)
            ones1 = cp.tile([1, 128], bf16)

            # startup order matters: only wq + b3 + chunk-0 x/cos-sin gate the
            # first projection; everything else is emitted behind them
            for t in range(8):
                nc.sync.dma_start(wq[:, t, :], wq_d[128 * t:128 * (t + 1), :])
            for i in range(3):
                nc.sync.dma_start(b3[:, i:i + 1], b3_d[i])
            nc.vector.memset(vall[:, :, 1, :], 1.0)
            nc.vector.memset(epsc[:], LN_EPS)
            nc.vector.memset(ones1[:], 1.0)

            a2a_in = dramp.tile([NCORES, 128, SLC], bf16)
            a2a_out = dramp.tile([NCORES, 128, SLC], bf16)

            with (
                tc.tile_pool(name="p1", bufs=1) as p1,
                tc.tile_pool(name="p2", bufs=1) as p2,
                tc.tile_pool(name="ps_sc", bufs=2, space="PSUM") as pssc,
                tc.tile_pool(name="ps_pj", bufs=2, space="PSUM") as pspj,
                tc.tile_pool(name="ps_ctx", bufs=1, space="PSUM") as psctx,
            ):
                def proj_chunk(w_sb, b_i, dst, isrope, ch):
                    ps = pspj.tile([128, 512], f32, tag="pj", name="psproj")
                    for t in range(8):
                        nc.tensor.matmul(
                            ps[:], w_sb[:, t, :], xtc[ch % 2][:, t, :],
                            start=(t == 0), stop=(t == 7))
                    if isrope:
                        # rot = (p+b)*cos_dup + (p_swapped+b)*sin_signed
                        mA = p1.tile([128, 512], f32, tag="mA", bufs=2)
                        mB = p1.tile([128, 512], f32, tag="mB", bufs=2)
                        bq0 = b3[0:64, b_i:b_i + 1]
                        bq1 = b3[64:128, b_i:b_i + 1]
                        csc_c = cscs[ch % 2]
                        nc.vector.scalar_tensor_tensor(
                            mA[:], ps[:], b3[:, b_i:b_i + 1], csc_c[:, 0, :],
                            op0=OP.add, op1=OP.mult)
                        nc.vector.scalar_tensor_tensor(
                            mB[0:64, :], ps[64:128, :], bq1, csc_c[64:128, 1, :],
                            op0=OP.add, op1=OP.mult)
                        nc.vector.scalar_tensor_tensor(
                            mB[64:128, :], ps[0:64, :], bq0, csc_c[0:64, 1, :],
                            op0=OP.add, op1=OP.mult)
                        nc.vector.tensor_add(dst[:], mA[:], mB[:])
                    else:
                        nc.vector.tensor_scalar_add(vtss[ch % 2][:], ps[:], b3[:, 2:3])

                def transpose_chunk(ch):
                    # V^T via PE transpose-mode (the DMA xbar transpose
                    # mis-transposes on real HW despite passing CoreSim)
                    for j in range(4):
                        st = 4 * ch + j
                        tp = pspj.tile([128, 128], bf16, tag="pj",
                                       padded_shape=[128, 1024], name="pstr")
                        nc.tensor.transpose(
                            tp[:], vtss[ch % 2][:, 128 * j:128 * (j + 1)],
                            ident[:])
                        nc.vector.tensor_copy(
                            vall[:, st, 0:3:2, :],
                            tp[:].rearrange("p (g c) -> p g c", c=64))

                def emit_qk(qb, kt):
                    # one group = one 128-wide k-tile against the q block.
                    # head A scores land in bank 0 of the sc slot, head B in
                    # bank 1; lhsT base partitions 0/64 row-pack the two
                    # matmuls into concurrent PE row-groups.
                    r = kt - NDIAG * qb
                    c0 = 128 * r if r > 0 else 0
                    sc = pssc.tile([128, 2, QB], f32, tag="sc", name="scsc")
                    pt = p2.tile([128, 2, QB], bf16, tag="pt", bufs=6)
                    kch, ko = kt // 4, 128 * (kt % 4)
                    nc.tensor.matmul(
                        sc[:, 0, c0:], k_sbs[kch][0:64, ko:ko + 128],
                        q_sbs[qb][0:64, c0:], start=True, stop=True)
                    nc.tensor.matmul(
                        sc[:, 1, c0:], k_sbs[kch][64:128, ko:ko + 128],
                        q_sbs[qb][64:128, c0:], start=True, stop=True)
                    nc.scalar.activation(pt[:, :, c0:], sc[:, :, c0:],
                                         AF.Exp, scale=0.125)
                    return pt

                def emit_pv(qb, kt, first, last, pt, ctx):
                    nk = (QB * (qb + 1)) // KT
                    r = kt - NDIAG * qb
                    c0 = 128 * r if r > 0 else 0
                    if r >= 0:  # diagonal k-tile: causal-mask the partial strip
                        cs_ = slice(128 * r, 128 * r + 128)
                        nc.vector.tensor_mul(pt[:, 0, cs_], pt[:, 0, cs_], tri[:])
                        nc.vector.tensor_mul(pt[:, 1, cs_], pt[:, 1, cs_], tri[:])
                    nc.tensor.matmul(
                        ctx[:, 0, c0:], vall[:, kt, 0:2, :], pt[:, 0, c0:],
                        start=(kt == 0), stop=(kt == nk - 1))
                    nc.tensor.matmul(
                        ctx[:, 1, c0:], vall[:, kt, 1:3, :], pt[:, 1, c0:],
                        start=(kt == 0), stop=(kt == nk - 1))

                def emit_norm(qb, ctx):
                    # head A: ctx rows 0:64, denom rows 64:128 (plane 0)
                    # head B: denom rows 0:64, ctx rows 64:128 (plane 1)
                    # denominators bounce through SBUF: the custom-DVE approx
                    # reciprocal mis-reads PSUM operands on real HW
                    qs = slice(QB * qb, QB * (qb + 1))
                    den = p2.tile([64, 2, QB], f32, tag="den", bufs=2)
                    rb = p2.tile([64, 2, QB], f32, tag="rb", bufs=2)
                    nc.vector.tensor_copy(den[:, 0, :], ctx[64:128, 0, :])
                    nc.vector.tensor_copy(den[:, 1, :], ctx[0:64, 1, :])
                    nc.vector.reciprocal_approx_fast(rb[:], den[:])
                    nc.vector.tensor_mul(ctxT[0:64, qs], ctx[0:64, 0, :], rb[:, 0, :])
                    nc.vector.tensor_mul(ctxT[64:128, qs], ctx[64:128, 1, :], rb[:, 1, :])

                def emit_bounce(qb):
                    # stage block qb's normalized ctx for the AllToAll as soon
                    # as its norm lands (hides the bounce behind attention)
                    for j in range(qb * QB // SLC, (qb + 1) * QB // SLC):
                        nc.gpsimd.dma_start(
                            a2a_in[j], ctxT[:, SLC * j:SLC * (j + 1)])

                def emit_phase4(qt):
                    # Wo + bias + bn-stats + LayerNorm for local rows
                    # [128*qt, 128*qt+128), normalized straight out of the two
                    # live PSUM slots (no SBUF staging copy).
                    stats = p1.tile([128, 2, 6], f32, tag="stats", bufs=2,
                                    name=f"stats{qt}")
                    tsl = slice(128 * qt, 128 * (qt + 1))
                    opss = []
                    for o in range(2):
                        osl = slice(512 * o, 512 * (o + 1))
                        if o == 0:
                            ops = pspj.tile([128, 512], f32, tag="pj",
                                            name="pswo")
                        else:
                            # attention is over: borrow the idle score-pool
                            # slots so 4 Wo groups pipeline instead of 2
                            ops = pssc.tile([128, 512], f32, tag="sc",
                                            padded_shape=[128, 1024],
                                            name="pswo2")
                        opss.append(ops)
                        for ct in range(8):
                            nc.tensor.matmul(
                                ops[:], ctxF[:, ct, tsl], wo[:, ct, osl],
                                start=(ct == 0), stop=False)
                        # rank-1 bias add closes the PSUM group
                        nc.tensor.matmul(
                            ops[:], ones1[:], bo16[:, osl],
                            start=False, stop=True)
                        nc.vector.bn_stats(stats[:, o, :], ops[:])
                    nc.vector.bn_aggr(mvs[:, qt, :], stats[:])
                    sd = p1.tile([128, 2], f32, tag="sd", bufs=2,
                                 name=f"sd{qt}")
                    nc.scalar.activation(sd[:, 0:1], mvs[:, qt, 1:2],
                                         AF.Sqrt, bias=epsc[:])
                    nc.vector.reciprocal(sd[:, 1:2], sd[:, 0:1])
                    t2 = p1.tile([128, DIM], f32, tag="t2", bufs=2,
                                 name=f"t2{qt}")
                    for o in range(2):
                        osl = slice(512 * o, 512 * (o + 1))
                        nc.vector.tensor_scalar(
                            t2[:, osl], opss[o][:], mvs[:, qt, 0:1], sd[:, 1:2],
                            op0=OP.subtract, op1=OP.mult)
                    if triv_ln:
                        nc.sync.dma_start(out_d[tsl, :], t2[:])
                    else:
                        t3 = p1.tile([128, DIM], f32, tag="t3", bufs=2,
                                     name=f"t3{qt}")
                        nc.vector.tensor_mul(t3[:], t2[:], lnc[:, 1, :])
                        ob = p1.tile([128, DIM], f32, tag="ob", bufs=2,
                                     name=f"ob{qt}")
                        nc.vector.tensor_add(ob[:], t3[:], lnc[:, 2, :])
                        nc.sync.dma_start(out_d[tsl, :], ob[:])

                xtc = [p1.tile([128, 8, 512], bf16, tag=f"xtc{s}", name=f"xtc{s}")
                       for s in range(2)]
                cscs = [p1.tile([128, 2, 512], bf16, tag=f"csc{s}", name=f"csc{s}")
                        for s in range(2)]
                vtss = [p1.tile([128, 512], bf16, tag=f"vts{s}", name=f"vts{s}")
                        for s in range(2)]

                from collections import deque
                pending = deque()   # (qb, kt, first, last, pt, ctx)

                def flush_pending():
                    while pending:
                        d = pending.popleft()
                        emit_pv(*d)
                        if d[3]:
                            emit_norm(d[0], d[5])

                ctx = None
                for ch in range(NCH):
                    for t in range(8):
                        eng = nc.sync if t % 2 == 0 else nc.scalar
                        eng.dma_start(
                            xtc[ch % 2][:, t, :],
                            xt_d[128 * t:128 * (t + 1), 512 * ch:512 * (ch + 1)])
                    for i in range(2):
                        nc.scalar.dma_start(
                            cscs[ch % 2][:, i, :], cs_d[i, :, 512 * ch:512 * (ch + 1)])
                    if ch == 0:
                        # deferred constants ride the idle gpsimd queue so the
                        # sync/scalar queues stay dedicated to the x stream
                        for t in range(8):
                            nc.gpsimd.dma_start(wk[:, t, :],
                                                wk_d[128 * t:128 * (t + 1), :])
                        for t in range(8):
                            nc.gpsimd.dma_start(wv[:, t, :],
                                                wv_d[128 * t:128 * (t + 1), :])
                        nc.sync.dma_start(tri[:], msk_d[:])
                        nc.sync.dma_start(ident[:], id_d[:])
                    if ch == 1:
                        # Wo / LN constants prefetch on the idle gpsimd queue
                        for t in range(8):
                            nc.gpsimd.dma_start(wo[:, t, :],
                                                wo_d[128 * t:128 * (t + 1), :])
                        nc.gpsimd.dma_start(bo16[:], bo16_d[:])
                        if not triv_ln:
                            for i in range(3):
                                nc.gpsimd.dma_start(lnc[:, i, :], lnc_d[i])

                    proj_chunk(wq, 0, q_sbs[ch], True, ch)
                    # drain leftovers of the previous block: PE gets PV work
                    # while the DVE computes this chunk's RoPE
                    flush_pending()
                    if ch >= 1:
                        emit_bounce(ch - 1)
                    proj_chunk(wk, 1, k_sbs[ch], True, ch)
                    proj_chunk(wv, 2, None, False, ch)
                    transpose_chunk(ch)

                    qb = ch
                    nk = (QB * (qb + 1)) // KT
                    for kt in range(nk):
                        if kt == 0:
                            ctx = psctx.tile([128, 2, QB], f32, tag="ctx")
                        pt = emit_qk(qb, kt)
                        pending.append((qb, kt, kt == 0, kt == nk - 1, pt, ctx))
                        if len(pending) > 2:
                            d = pending.popleft()
                            emit_pv(*d)
                            if d[3]:
                                emit_norm(d[0], d[5])

                flush_pending()
                emit_bounce(NQB - 1)
                nc.gpsimd.collective_compute(
                    "AllToAll",
                    mybir.AluOpType.bypass,
                    replica_groups=[list(range(NCORES))],
                    ins=[a2a_in[:].opt()],
                    outs=[a2a_out[:].opt()],
                )
                ctxF = p1.tile([128, NCORES, SLC], bf16)
                for j in range(NCORES):
                    nc.sync.dma_start(ctxF[:, j, :], a2a_out[j])

                for qt in range(SLC // 128):
                    emit_phase4(qt)

    nc.compile()
    return nc


def get_nc(S=4096, triv_ln=False):
    key = (S, triv_ln)
    if key not in _built:
        _built[key] = _build(S, triv_ln)
    return _built[key]


def stage_inputs(x, Wqkv, bqkv, Wo, bo, gamma, beta):
    """Host-side sharding/staging. Returns in_maps for the 8 cores."""
    x = np.asarray(x, dtype=np.float32)
    Wqkv = np.asarray(Wqkv, dtype=np.float32)
    bqkv = np.asarray(bqkv, dtype=np.float32)
    Wo = np.asarray(Wo, dtype=np.float32)
    bo = np.asarray(bo, dtype=np.float32)
    gamma = np.asarray(gamma, dtype=np.float32)
    beta = np.asarray(beta, dtype=np.float32)

    S = x.shape[1]
    xt = np.ascontiguousarray(x[0].T).astype(BF)                       # [DIM, S]
    inv_freq = 1.0 / (ROPE_THETA ** (np.arange(0, DIM, 2, dtype=np.float64) / DIM))

    # Wo rows permuted to the post-AllToAll channel order
    perm = np.concatenate([
        np.concatenate([np.arange(64 * j, 64 * j + 64),
                        np.arange(512 + 64 * j, 512 + 64 * j + 64)])
        for j in range(NCORES)
    ])
    wo = np.ascontiguousarray(Wo[perm, :]).astype(BF)

    p = np.arange(128)[:, None]
    u = np.arange(128)[None, :]
    tri = (p <= u).astype(BF)                     # causal mask for diag strips
    ident = np.eye(128, dtype=np.float32).astype(BF)
    lnc = np.stack([
        np.broadcast_to(bo, (128, DIM)),
        np.broadcast_to(gamma, (128, DIM)),
        np.broadcast_to(beta, (128, DIM)),
    ]).astype(np.float32)

    in_maps = []
    for c in range(NCORES):
        cols = np.concatenate([np.arange(64 * c, 64 * c + 64),
                               np.arange(512 + 64 * c, 512 + 64 * c + 64)])
        ang = np.arange(S, dtype=np.float64)[None, :] * inv_freq[64 * c:64 * c + 64][:, None]
        C = np.cos(ang)
        Sn = np.sin(ang)
        # plane 0: cos duplicated; plane 1: +sin rows 0:64, -sin rows 64:128
        # (the sign flip folds the rotate-half subtraction into one tensor_add)
        cs = np.stack([np.concatenate([C, C], 0),
                       np.concatenate([Sn, -Sn], 0)]).astype(BF)        # [2,128,S]
        b3 = np.stack([bqkv[cols], bqkv[1024 + cols], bqkv[2048 + cols]]
                      ).astype(np.float32)[:, :, None]                  # [3,128,1]
        in_maps.append({
            "xt": xt,
            "wq": np.ascontiguousarray(Wqkv[:, cols]).astype(BF),
            "wk": np.ascontiguousarray(Wqkv[:, 1024 + cols]).astype(BF),
            "wv": np.ascontiguousarray(Wqkv[:, 2048 + cols]).astype(BF),
            "b3": b3,
            "cs": cs,
            "msk": tri,
            "ident": ident,
            "wo": wo,
            "bo16": bo.reshape(1, DIM).astype(BF),
            "lnc": lnc,
        })
    return in_maps


def gather_out(outs, S):
    """Reassemble the full [1, S, DIM] output: core c owns rows
    [S/8*c : S/8*(c+1))."""
    return np.concatenate(outs, axis=0)[None]


def kernel(x, Wqkv, bqkv, Wo, bo, gamma, beta):
    from concourse import bass_utils

    x = np.asarray(x)
    S = x.shape[1]
    triv = bool(np.all(np.asarray(gamma) == 1.0) and np.all(np.asarray(beta) == 0.0))
    nc = get_nc(S, triv)
    in_maps = stage_inputs(x, Wqkv, bqkv, Wo, bo, gamma, beta)
    res = bass_utils.run_bass_kernel_spmd(nc, in_maps, core_ids=list(range(NCORES)))
    return gather_out([res.results[c]["out"] for c in range(NCORES)], S).astype(
        np.float32)


# revision 22
# speedup vs baseline: 1.1315x; 1.0671x over previous
"""Distributed Trainium2 Bass kernel for nn_AttentionCell (B=1, S=4096, D=1024, H=16).

Sharding: tensor-parallel over heads, 2 heads per core, paired (h, h+8) so RoPE's
rotate-half (which pairs model dims d and d+512, i.e. heads h and h+8) stays local
to a core. Per core:
  - QKV projection for its 128 channels (computed transposed: [ch, S]) from a
    host-staged transposed bf16 copy of x.
  - RoPE via host-staged cos/sin tables fused with the bias-add on the
    PSUM->SBUF drain.
  - Causal attention with scores computed transposed ([k, q] layout) so the
    PV matmul needs no on-chip transposes; softmax without max-subtraction
    (scores are O(1) here, exp cannot overflow); denominator via ones-columns
    appended to V (PV output rows 64:128); diagonal-crossing k-tiles are
    column-range-restricted so exp/QK/PV skip the fully-masked region.
  - One AllToAll (bounce slices staged per block as attention completes)
    switches to sequence-parallel, then the output projection + bias +
    LayerNorm run on this core's S/8 rows. (Chunked/warmup collectives and
    sub-256KB peer slices silently mis-route between core pairs on this
    runtime - only the single late AllToAll with [128,512]-bf16 peer slices
    is reliable.)
Host concatenates the 8 per-core row-slices into the full output.
"""
import os
import sys

sys.path.insert(0, "/opt/trn_rl_repo")

import numpy as np
import ml_dtypes

BF = ml_dtypes.bfloat16

DIM = 1024
H = 16
NCORES = 8
QB = 512          # query block (columns of transposed scores)
KT = 128          # key tile (partition dim of transposed scores)
NDIAG = QB // KT  # k-tiles crossing the causal diagonal per q block
GR = 128          # output-row granule for the chunked AllToAll
LN_EPS = 1e-5
ROPE_THETA = 10000.0


_built = {}


def _build(S, triv_ln=False):
    """Build + compile the 8-core SPMD graph for sequence length S."""
    import concourse.bass as bass
    import concourse.bacc as bacc
    import concourse.tile as tile
    import concourse.mybir as mybir

    f32 = mybir.dt.float32
    bf16 = mybir.dt.bfloat16
    AF = mybir.ActivationFunctionType
    OP = mybir.AluOpType

    assert S % 1024 == 0 and (S // NCORES) % 128 == 0
    SLC = S // NCORES          # output rows per core
    NQB = S // QB              # number of query blocks (= chunks)
    NKT = S // KT              # number of key tiles
    NCH = S // 512             # 512-wide chunks for projections (1 block each)
    NA2A = NQB // 2            # AllToAll chunks (2 blocks each)

    nc = bacc.Bacc("TRN2", target_bir_lowering=False, debug=False, num_devices=NCORES)

    xt_d = nc.dram_tensor("xt", [DIM, S], bf16, kind="ExternalInput").ap()
    wq_d = nc.dram_tensor("wq", [DIM, 128], bf16, kind="ExternalInput").ap()
    wk_d = nc.dram_tensor("wk", [DIM, 128], bf16, kind="ExternalInput").ap()
    wv_d = nc.dram_tensor("wv", [DIM, 128], bf16, kind="ExternalInput").ap()
    b3_d = nc.dram_tensor("b3", [3, 128, 1], f32, kind="ExternalInput").ap()
    cs_d = nc.dram_tensor("cs", [2, 128, S], bf16, kind="ExternalInput").ap()
    msk_d = nc.dram_tensor("msk", [128, 128], bf16, kind="ExternalInput").ap()
    id_d = nc.dram_tensor("ident", [128, 128], bf16, kind="ExternalInput").ap()
    wo_d = nc.dram_tensor("wo", [DIM, DIM], bf16, kind="ExternalInput").ap()
    bo16_d = nc.dram_tensor("bo16", [1, DIM], bf16, kind="ExternalInput").ap()
    lnc_d = nc.dram_tensor("lnc", [3, 128, DIM], f32, kind="ExternalInput").ap()
    out_d = nc.dram_tensor("out", [SLC, DIM], f32, kind="ExternalOutput").ap()

    with tile.TileContext(nc) as tc:
        with (
            tc.tile_pool(name="const", bufs=1) as cp,
            tc.tile_pool(name="dram", bufs=1, space="DRAM") as dramp,
        ):
            wq = cp.tile([128, 8, 128], bf16)
            wk = cp.tile([128, 8, 128], bf16)
            wv = cp.tile([128, 8, 128], bf16)
            b3 = cp.tile([128, 3], f32)
            tri = cp.tile([128, 128], bf16)     # tri[p, u] = (p <= u)
            ident = cp.tile([128, 128], bf16)
            wo = cp.tile([128, 8, DIM], bf16)
            lnc = cp.tile([128, 3, DIM], f32)
            bo16 = cp.tile([1, DIM], bf16)
            q_sbs = [cp.tile([128, 512], bf16, name=f"qsb{c}") for c in range(NCH)]
            k_sbs = [cp.tile([128, 512], bf16, name=f"ksb{c}") for c in range(NCH)]
            # per-k-tile [V_A(64) | ones(64) | V_B(64)]: the shared ones
            # plane gives both heads a contiguous 128-col PV lhsT
            # ([V_A|ones], [ones|V_B]) whose ones half emits the softmax
            # denominator (head A: ctx rows 64:128; head B: ctx rows 0:64).
            vall = cp.tile([128, NKT, 3, 64], bf16)
            ctxT = cp.tile([128, S], bf16)      # normalized ctx, [ch, q]
            mvs = cp.tile([128, SLC // 128, 2], f32)       # per-qtile LN mean/var
            epsc = cp.tile([128, 1], f32)
            ones1 = cp.tile([1, 128], bf16)

            # startup order matters: only wq + b3 + chunk-0 x/cos-sin gate the
            # first projection; everything else is emitted behind them.
            # (chunk 0's x tiles are interleaved with wq below in the chunk
            # loop via subtile deps - emit only wq/b3 here)
            for t in range(8):
                nc.sync.dma_start(wq[:, t, :], wq_d[128 * t:128 * (t + 1), :])
            for i in range(3):
                nc.sync.dma_start(b3[:, i:i + 1], b3_d[i])
            nc.vector.memset(vall[:, :, 1, :], 1.0)
            nc.vector.memset(epsc[:], LN_EPS)
            nc.vector.memset(ones1[:], 1.0)

            a2a_in = dramp.tile([NCORES, 128, SLC], bf16)
            a2a_out = dramp.tile([NCORES, 128, SLC], bf16)

            with (
                tc.tile_pool(name="p1", bufs=1) as p1,
                tc.tile_pool(name="p2", bufs=1) as p2,
                tc.tile_pool(name="ps_sc", bufs=2, space="PSUM") as pssc,
                tc.tile_pool(name="ps_pj", bufs=2, space="PSUM") as pspj,
                tc.tile_pool(name="ps_ctx", bufs=1, space="PSUM") as psctx,
            ):
                def proj_chunk(w_sb, b_i, dst, isrope, ch):
                    ps = pspj.tile([128, 512], f32, tag="pj", name="psproj")
                    for t in range(8):
                        nc.tensor.matmul(
                            ps[:], w_sb[:, t, :], xtc[ch % 2][:, t, :],
                            start=(t == 0), stop=(t == 7))
                    if isrope:
                        # rot = (p+b)*cos_dup + (p_swapped+b)*sin_signed
                        mA = p1.tile([128, 512], f32, tag="mA", bufs=2)
                        mB = p1.tile([128, 512], f32, tag="mB", bufs=2)
                        bq0 = b3[0:64, b_i:b_i + 1]
                        bq1 = b3[64:128, b_i:b_i + 1]
                        csc_c = cscs[ch % 2]
                        nc.vector.scalar_tensor_tensor(
                            mA[:], ps[:], b3[:, b_i:b_i + 1], csc_c[:, 0, :],
                            op0=OP.add, op1=OP.mult)
                        nc.vector.scalar_tensor_tensor(
                            mB[0:64, :], ps[64:128, :], bq1, csc_c[64:128, 1, :],
                            op0=OP.add, op1=OP.mult)
                        nc.vector.scalar_tensor_tensor(
                            mB[64:128, :], ps[0:64, :], bq0, csc_c[0:64, 1, :],
                            op0=OP.add, op1=OP.mult)
                        nc.vector.tensor_add(dst[:], mA[:], mB[:])
                    else:
                        nc.vector.tensor_scalar_add(vtss[ch % 2][:], ps[:], b3[:, 2:3])

                def transpose_chunk(ch):
                    # V^T via PE transpose-mode (the DMA xbar transpose
                    # mis-transposes on real HW despite passing CoreSim)
                    for j in range(4):
                        st = 4 * ch + j
                        tp = pspj.tile([128, 128], bf16, tag="pj",
                                       padded_shape=[128, 1024], name="pstr")
                        nc.tensor.transpose(
                            tp[:], vtss[ch % 2][:, 128 * j:128 * (j + 1)],
                            ident[:])
                        nc.vector.tensor_copy(
                            vall[:, st, 0:3:2, :],
                            tp[:].rearrange("p (g c) -> p g c", c=64))

                def emit_qk(qb, kt):
                    # one group = one 128-wide k-tile against the q block.
                    # head A scores land in bank 0 of the sc slot, head B in
                    # bank 1; lhsT base partitions 0/64 row-pack the two
                    # matmuls into concurrent PE row-groups.
                    r = kt - NDIAG * qb
                    c0 = 128 * r if r > 0 else 0
                    sc = pssc.tile([128, 2, QB], f32, tag="sc", name="scsc")
                    pt = p2.tile([128, 2, QB], bf16, tag="pt", bufs=6)
                    kch, ko = kt // 4, 128 * (kt % 4)
                    nc.tensor.matmul(
                        sc[:, 0, c0:], k_sbs[kch][0:64, ko:ko + 128],
                        q_sbs[qb][0:64, c0:], start=True, stop=True)
                    nc.tensor.matmul(
                        sc[:, 1, c0:], k_sbs[kch][64:128, ko:ko + 128],
                        q_sbs[qb][64:128, c0:], start=True, stop=True)
                    nc.scalar.activation(pt[:, :, c0:], sc[:, :, c0:],
                                         AF.Exp, scale=0.125)
                    return pt

                def emit_pv(qb, kt, first, last, pt, ctx):
                    nk = (QB * (qb + 1)) // KT
                    r = kt - NDIAG * qb
                    c0 = 128 * r if r > 0 else 0
                    if r >= 0:  # diagonal k-tile: causal-mask the partial strip
                        cs_ = slice(128 * r, 128 * r + 128)
                        nc.vector.tensor_mul(pt[:, 0, cs_], pt[:, 0, cs_], tri[:])
                        nc.vector.tensor_mul(pt[:, 1, cs_], pt[:, 1, cs_], tri[:])
                    nc.tensor.matmul(
                        ctx[:, 0, c0:], vall[:, kt, 0:2, :], pt[:, 0, c0:],
                        start=(kt == 0), stop=(kt == nk - 1))
                    nc.tensor.matmul(
                        ctx[:, 1, c0:], vall[:, kt, 1:3, :], pt[:, 1, c0:],
                        start=(kt == 0), stop=(kt == nk - 1))

                def emit_norm(qb, ctx):
                    # head A: ctx rows 0:64, denom rows 64:128 (plane 0)
                    # head B: denom rows 0:64, ctx rows 64:128 (plane 1)
                    # denominators bounce through SBUF: the custom-DVE approx
                    # reciprocal mis-reads PSUM operands on real HW
                    qs = slice(QB * qb, QB * (qb + 1))
                    den = p2.tile([64, 2, QB], f32, tag="den", bufs=2)
                    rb = p2.tile([64, 2, QB], f32, tag="rb", bufs=2)
                    nc.vector.tensor_copy(den[:, 0, :], ctx[64:128, 0, :])
                    nc.vector.tensor_copy(den[:, 1, :], ctx[0:64, 1, :])
                    nc.vector.reciprocal_approx_fast(rb[:], den[:])
                    nc.vector.tensor_mul(ctxT[0:64, qs], ctx[0:64, 0, :], rb[:, 0, :])
                    nc.vector.tensor_mul(ctxT[64:128, qs], ctx[64:128, 1, :], rb[:, 1, :])

                def emit_bounce(qb):
                    # stage block qb's normalized ctx for the AllToAll as soon
                    # as its norm lands (hides the bounce behind attention)
                    for j in range(qb * QB // SLC, (qb + 1) * QB // SLC):
                        nc.gpsimd.dma_start(
                            a2a_in[j], ctxT[:, SLC * j:SLC * (j + 1)])

                def emit_phase4(qt):
                    # Wo + bias + bn-stats + LayerNorm for local rows
                    # [128*qt, 128*qt+128), normalized straight out of the two
                    # live PSUM slots (no SBUF staging copy).
                    stats = p1.tile([128, 2, 6], f32, tag="stats", bufs=2,
                                    name=f"stats{qt}")
                    tsl = slice(128 * qt, 128 * (qt + 1))
                    opss = []
                    for o in range(2):
                        osl = slice(512 * o, 512 * (o + 1))
                        if o == 0:
                            ops = pspj.tile([128, 512], f32, tag="pj",
                                            name="pswo")
                        else:
                            # attention is over: borrow the idle score-pool
                            # slots so 4 Wo groups pipeline instead of 2
                            ops = pssc.tile([128, 512], f32, tag="sc",
                                            padded_shape=[128, 1024],
                                            name="pswo2")
                        opss.append(ops)
                        for ct in range(8):
                            nc.tensor.matmul(
                                ops[:], ctxF[:, ct, tsl], wo[:, ct, osl],
                                start=(ct == 0), stop=False)
                        # rank-1 bias add closes the PSUM group
                        nc.tensor.matmul(
                            ops[:], ones1[:], bo16[:, osl],
                            start=False, stop=True)
                        nc.vector.bn_stats(stats[:, o, :], ops[:])
                    nc.vector.bn_aggr(mvs[:, qt, :], stats[:])
                    sd = p1.tile([128, 2], f32, tag="sd", bufs=2,
                                 name=f"sd{qt}")
                    nc.scalar.activation(sd[:, 0:1], mvs[:, qt, 1:2],
                                         AF.Sqrt, bias=epsc[:])
                    nc.vector.reciprocal(sd[:, 1:2], sd[:, 0:1])
                    t2 = p1.tile([128, DIM], f32, tag="t2", bufs=2,
                                 name=f"t2{qt}")
                    for o in range(2):
                        osl = slice(512 * o, 512 * (o + 1))
                        nc.vector.tensor_scalar(
                            t2[:, osl], opss[o][:], mvs[:, qt, 0:1], sd[:, 1:2],
                            op0=OP.subtract, op1=OP.mult)
                    if triv_ln:
                        nc.sync.dma_start(out_d[tsl, :], t2[:])
                    else:
                        t3 = p1.tile([128, DIM], f32, tag="t3", bufs=2,
                                     name=f"t3{qt}")
                        nc.vector.tensor_mul(t3[:], t2[:], lnc[:, 1, :])
                        ob = p1.tile([128, DIM], f32, tag="ob", bufs=2,
                                     name=f"ob{qt}")
                        nc.vector.tensor_add(ob[:], t3[:], lnc[:, 2, :])
                        nc.sync.dma_start(out_d[tsl, :], ob[:])

                xtc = [p1.tile([128, 8, 512], bf16, tag=f"xtc{s}", name=f"xtc{s}")
                       for s in range(2)]
                cscs = [p1.tile([128, 2, 512], bf16, tag=f"csc{s}", name=f"csc{s}")
                        for s in range(2)]
                vtss = [p1.tile([128, 512], bf16, tag=f"vts{s}", name=f"vts{s}")
                        for s in range(2)]

                from collections import deque
                pending = deque()   # (qb, kt, first, last, pt, ctx)

                def flush_pending():
                    while pending:
                        d = pending.popleft()
                        emit_pv(*d)
                        if d[3]:
                            emit_norm(d[0], d[5])

                ctx = None
                for ch in range(NCH):
                    for t in range(8):
                        eng = nc.sync if t % 2 == 0 else nc.scalar
                        eng.dma_start(
                            xtc[ch % 2][:, t, :],
                            xt_d[128 * t:128 * (t + 1), 512 * ch:512 * (ch + 1)])
                    for i in range(2):
                        nc.scalar.dma_start(
                            cscs[ch % 2][:, i, :], cs_d[i, :, 512 * ch:512 * (ch + 1)])
                    if ch == 0:
                        # deferred constants ride the idle gpsimd queue so the
                        # sync/scalar queues stay dedicated to the x stream
                        for t in range(8):
                            nc.gpsimd.dma_start(wk[:, t, :],
                                                wk_d[128 * t:128 * (t + 1), :])
                        for t in range(8):
                            nc.gpsimd.dma_start(wv[:, t, :],
                                                wv_d[128 * t:128 * (t + 1), :])
                        nc.sync.dma_start(tri[:], msk_d[:])
                        nc.sync.dma_start(ident[:], id_d[:])
                    if ch == 1:
                        # Wo / LN constants prefetch on the idle gpsimd queue
                        for t in range(8):
                            nc.gpsimd.dma_start(wo[:, t, :],
                                                wo_d[128 * t:128 * (t + 1), :])
                        nc.gpsimd.dma_start(bo16[:], bo16_d[:])
                        if not triv_ln:
                            for i in range(3):
                                nc.gpsimd.dma_start(lnc[:, i, :], lnc_d[i])

                    proj_chunk(wq, 0, q_sbs[ch], True, ch)
                    # drain leftovers of the previous block: PE gets PV work
                    # while the DVE computes this chunk's RoPE
                    flush_pending()
                    if ch >= 1:
                        emit_bounce(ch - 1)
                    proj_chunk(wk, 1, k_sbs[ch], True, ch)
                    proj_chunk(wv, 2, None, False, ch)
                    transpose_chunk(ch)

                    qb = ch
                    nk = (QB * (qb + 1)) // KT
                    for kt in range(nk):
                        if kt == 0:
                            ctx = psctx.tile([128, 2, QB], f32, tag="ctx")
                        pt = emit_qk(qb, kt)
                        pending.append((qb, kt, kt == 0, kt == nk - 1, pt, ctx))
                        if len(pending) > 2:
                            d = pending.popleft()
                            emit_pv(*d)
                            if d[3]:
                                emit_norm(d[0], d[5])

                flush_pending()
                emit_bounce(NQB - 1)
                nc.gpsimd.collective_compute(
                    "AllToAll",
                    mybir.AluOpType.bypass,
                    replica_groups=[list(range(NCORES))],
                    ins=[a2a_in[:].opt()],
                    outs=[a2a_out[:].opt()],
                )
                ctxF = p1.tile([128, NCORES, SLC], bf16)
                for j in range(NCORES):
                    eng = nc.sync if j % 2 == 0 else nc.scalar
                    eng.dma_start(ctxF[:, j, :], a2a_out[j])

                for qt in range(SLC // 128):
                    emit_phase4(qt)

    nc.compile()
    return nc


def get_nc(S=4096, triv_ln=False):
    key = (S, triv_ln)
    if key not in _built:
        _built[key] = _build(S, triv_ln)
    return _built[key]


def stage_inputs(x, Wqkv, bqkv, Wo, bo, gamma, beta):
    """Host-side sharding/staging. Returns in_maps for the 8 cores."""
    x = np.asarray(x, dtype=np.float32)
    Wqkv = np.asarray(Wqkv, dtype=np.float32)
    bqkv = np.asarray(bqkv, dtype=np.float32)
    Wo = np.asarray(Wo, dtype=np.float32)
    bo = np.asarray(bo, dtype=np.float32)
    gamma = np.asarray(gamma, dtype=np.float32)
    beta = np.asarray(beta, dtype=np.float32)

    S = x.shape[1]
    xt = np.ascontiguousarray(x[0].T).astype(BF)                       # [DIM, S]
    inv_freq = 1.0 / (ROPE_THETA ** (np.arange(0, DIM, 2, dtype=np.float64) / DIM))

    # Wo rows permuted to the post-AllToAll channel order
    perm = np.concatenate([
        np.concatenate([np.arange(64 * j, 64 * j + 64),
                        np.arange(512 + 64 * j, 512 + 64 * j + 64)])
        for j in range(NCORES)
    ])
    wo = np.ascontiguousarray(Wo[perm, :]).astype(BF)

    p = np.arange(128)[:, None]
    u = np.arange(128)[None, :]
    tri = (p <= u).astype(BF)                     # causal mask for diag strips
    ident = np.eye(128, dtype=np.float32).astype(BF)
    lnc = np.stack([
        np.broadcast_to(bo, (128, DIM)),
        np.broadcast_to(gamma, (128, DIM)),
        np.broadcast_to(beta, (128, DIM)),
    ]).astype(np.float32)

    in_maps = []
    for c in range(NCORES):
        cols = np.concatenate([np.arange(64 * c, 64 * c + 64),
                               np.arange(512 + 64 * c, 512 + 64 * c + 64)])
        ang = np.arange(S, dtype=np.float64)[None, :] * inv_freq[64 * c:64 * c + 64][:, None]
        C = np.cos(ang)
        Sn = np.sin(ang)
        # plane 0: cos duplicated; plane 1: +sin rows 0:64, -sin rows 64:128
        # (the sign flip folds the rotate-half subtraction into one tensor_add)
        cs = np.stack([np.concatenate([C, C], 0),
                       np.concatenate([Sn, -Sn], 0)]).astype(BF)        # [2,128,S]
        b3 = np.stack([bqkv[cols], bqkv[1024 + cols], bqkv[2048 + cols]]
                      ).astype(np.float32)[:, :, None]                  # [3,128,1]
        in_maps.append({
            "xt": xt,
            "wq": np.ascontiguousarray(Wqkv[:, cols]).astype(BF),
            "wk": np.ascontiguousarray(Wqkv[:, 1024 + cols]).astype(BF),
            "wv": np.ascontiguousarray(Wqkv[:, 2048 + cols]).astype(BF),
            "b3": b3,
            "cs": cs,
            "msk": tri,
            "ident": ident,
            "wo": wo,
            "bo16": bo.reshape(1, DIM).astype(BF),
            "lnc": lnc,
        })
    return in_maps


def gather_out(outs, S):
    """Reassemble the full [1, S, DIM] output: core c owns rows
    [S/8*c : S/8*(c+1))."""
    return np.concatenate(outs, axis=0)[None]


def kernel(x, Wqkv, bqkv, Wo, bo, gamma, beta):
    from concourse import bass_utils

    x = np.asarray(x)
    S = x.shape[1]
    triv = bool(np.all(np.asarray(gamma) == 1.0) and np.all(np.asarray(beta) == 0.0))
    nc = get_nc(S, triv)
    in_maps = stage_inputs(x, Wqkv, bqkv, Wo, bo, gamma, beta)
    res = bass_utils.run_bass_kernel_spmd(nc, in_maps, core_ids=list(range(NCORES)))
    return gather_out([res.results[c]["out"] for c in range(NCORES)], S).astype(
        np.float32)
